# revision 3
# baseline (speedup 1.0000x reference)
"""BitLinear forward, tensor-parallel over 8 NeuronCores.

Sharding as v1 (weight rows split 8 ways, x replicated, host concat).

Schedule:
- W1: pure dual-queue (gpsimd+sync) streaming of the 32MB w shard through
  a 3-tile ring, |w| row sums on DVE. No other DMA traffic in this window.
- alpha AllReduce issued on the sync queue (idle during the collective);
  gpsimd/vector/scalar keep flowing: x-quant chains 0-2 (x loads on the
  scalar queue) and the first W2 reloads prefetch during the CC wait.
- W2: quantize on DVE (magic-round mul, clip, fused min+fp8 convert),
  transposes on sync, loads alternating gpsimd/vector, into a kt-major
  fp8 layout (every matmul moving operand is a contiguous [128,512]).
- Matmuls start when o-tiles 0-3 are ready; the first 3 row-tiles run
  ob-major (sweep o-blocks across tts) so consumption tracks W2's
  o-tile arrival order with no stalls.
- Output stored as bf16 (halves store traffic; host upcasts — the
  rounding is ~2^-9 rel, far under the 2e-2 gate).
"""

import numpy as np

import concourse.bass as bass
import concourse.mybir as mybir
import concourse.tile as tile
from concourse.bass_utils import run_bass_kernel_spmd


MAX_ATTACHED_WAITS = 1


def _split_sync_waits(nc, max_waits=MAX_ATTACHED_WAITS):
    nhoisted = 0
    for f in nc.m.functions:
        for blk in f.blocks:
            out = []
            changed = False
            for inst in blk.instructions:
                si = inst.sync_info
                if si is not None and len(si.on_wait) > max_waits:
                    waits = list(si.on_wait)
                    for wt in waits[max_waits:]:
                        out.append(
                            mybir.InstNoOp(
                                name=f"syncsplit-{nc.next_id()}",
                                ins=[],
                                outs=[],
                                engine=inst.engine,
                                sync_info=mybir.SyncInfo(
                                    on_wait=[wt], on_update=[]
                                ),
                                bass_nofuse=True,
                            )
                        )
                        nhoisted += 1
                    inst.sync_info = mybir.SyncInfo(
                        on_wait=waits[:max_waits], on_update=list(si.on_update)
                    )
                    changed = True
                out.append(inst)
            if changed:
                blk.instructions = out
    return nhoisted


F32 = mybir.dt.float32
BF16 = mybir.dt.bfloat16
FP8 = mybir.dt.float8e4

MAGIC = 1.5 * 2.0**23  # add/sub rounds f32 to nearest int (ties to even)
EPS = 1e-6

N_CORES = 8
AFT = mybir.ActivationFunctionType
ALU = mybir.AluOpType


def build(T, K, O, n_cores, use_nw, ring=3, nbank=3, pre=3, npA=3):
    """One-core SPMD program: x[T,K] f32, w[O,K] f32 shard, nw[1,K] -> y[T,O]."""
    TT, KT, OT = T // 128, K // 128, O // 128
    OBN = max(1, O // 512)
    OBW = O // OBN
    assert OBW <= 512
    assert pre == nbank

    nc = bass.Bass(
        "TRN2", target_bir_lowering=False, debug=False, num_devices=n_cores
    )
    x = nc.dram_tensor("x", [T, K], F32, kind="ExternalInput")
    w = nc.dram_tensor("w", [O, K], F32, kind="ExternalInput")
    nw = nc.dram_tensor("nw", [1, K], F32, kind="ExternalInput")
    y = nc.dram_tensor("y", [T, O], BF16, kind="ExternalOutput")

    inv_count = 1.0 / (O * n_cores * K)

    with tile.TileContext(nc) as tc:
        with (
            tc.tile_pool(name="const", bufs=1) as cpool,
            tc.tile_pool(name="wres", bufs=1) as wres,
            tc.tile_pool(name="wstage", bufs=ring) as wstage,
            tc.tile_pool(name="xstage", bufs=2) as xstage,
            tc.tile_pool(name="xq", bufs=2) as xqp,
            tc.tile_pool(name="bank", bufs=nbank) as bankp,
            tc.tile_pool(name="w16", bufs=1) as w16p,
            tc.tile_pool(name="osb", bufs=2) as osbp,
            tc.tile_pool(name="stat", bufs=4) as spool,
            tc.tile_pool(name="psum", bufs=8, space="PSUM") as ps,
            tc.tile_pool(name="dram", bufs=1, space="DRAM") as dram,
        ):
            # ---- constants ----
            posmagic = cpool.tile([128, 1], F32, tag="posmagic")
            nc.vector.memset(posmagic[:], MAGIC)
            epsb = cpool.tile([128, 1], F32, tag="epsb")
            nc.vector.memset(epsb[:], EPS)
            ones_col = cpool.tile([128, 1], F32, tag="ones_col")
            nc.vector.memset(ones_col[:], 1.0)
            ones_row = cpool.tile([1, 128], F32, tag="ones_row")
            nc.vector.memset(ones_row[:], 1.0)
            alpha_bc = cpool.tile([128, 1], F32, tag="alpha_bc")
            inv_alpha_bc = cpool.tile([128, 1], F32, tag="inv_alpha_bc")
            wsum = cpool.tile([128, OT], F32, tag="wsum")

            if use_nw:
                nw_rep = cpool.tile([128, K], F32, tag="nw_rep")
                nc.gpsimd.dma_start(nw_rep[0:1, :], nw.ap())
                p = 1
                while p < 128:
                    nc.gpsimd.dma_start(nw_rep[p : 2 * p, :], nw_rep[0:p, :])
                    p *= 2

            # resident fp8 weights, kt-major: column kt*O + o holds
            # w_q[o, kt*128 + p]
            wqs = wres.tile([128, KT * O], FP8, tag="wqs")

            sys_ = {}
            banks = {}
            sy_tiles = {}

            def emit_sy(tt):
                gor = sys_.pop(tt)
                sy = spool.tile([128, 1], F32, tag="sy", name=f"sy_{tt}")
                nc.vector.tensor_scalar(
                    out=sy[:],
                    in0=gor[:],
                    scalar1=alpha_bc[:],
                    scalar2=1.0 / 127.0,
                    op0=ALU.mult,
                    op1=ALU.mult,
                )
                sy_tiles[tt] = sy

            def chain(tt, with_sy):
                xin = xstage.tile([128, K], F32, tag="xin", name=f"xin_{tt}")
                nc.scalar.dma_start(xin[:], x[tt * 128 : (tt + 1) * 128, :])

                bank_t = bankp.tile([128, K], BF16, tag="bank", name=f"bank_{tt}")
                ss = spool.tile([128, 1], F32, tag="ss", name=f"ss_{tt}")
                # elementwise square output is scratch; dump into the bank
                # tile (overwritten below by the transpose)
                nc.scalar.activation(
                    bank_t[:], xin[:], AFT.Square, accum_out=ss[:]
                )

                if use_nw:
                    u = xstage.tile([128, K], F32, tag="xin", name=f"u_{tt}")
                    nc.vector.tensor_mul(u[:], xin[:], nw_rep[:])
                    src = u
                else:
                    src = xin

                graw = spool.tile([128, 1], F32, tag="graw", name=f"graw_{tt}")
                nc.vector.tensor_reduce(
                    graw[:],
                    src[:],
                    axis=mybir.AxisListType.X,
                    op=ALU.max,
                    apply_absolute_value=True,
                )
                g = spool.tile([128, 1], F32, tag="g", name=f"g_{tt}")
                nc.vector.tensor_scalar_max(g[:], graw[:], 1e-10)
                invg = spool.tile([128, 1], F32, tag="invg", name=f"invg_{tt}")
                nc.vector.reciprocal(invg[:], g[:])
                s127 = spool.tile([128, 1], F32, tag="s127", name=f"s127_{tt}")
                nc.vector.tensor_scalar_mul(s127[:], invg[:], 127.0)
                rms = spool.tile([128, 1], F32, tag="rms", name=f"rms_{tt}")
                nc.scalar.activation(
                    rms[:], ss[:], AFT.Sqrt, bias=epsb[:], scale=1.0 / K
                )
                invrms = spool.tile([128, 1], F32, tag="invrms", name=f"invrms_{tt}")
                nc.vector.reciprocal(invrms[:], rms[:])
                gor = spool.tile([128, 1], F32, tag="gor", name=f"gor_{tt}")
                nc.vector.tensor_mul(gor[:], g[:], invrms[:])
                sys_[tt] = gor

                # q1 = src*127/g + MAGIC, in place over the staging tile
                nc.vector.tensor_scalar(
                    out=src[:],
                    in0=src[:],
                    scalar1=s127[:],
                    scalar2=MAGIC,
                    op0=ALU.mult,
                    op1=ALU.add,
                )
                xq = xqp.tile([128, K], BF16, tag="xq", name=f"xq_{tt}")
                nc.vector.tensor_scalar_add(xq[:], src[:], -MAGIC)

                # transpose into the bank tile; scalar queue (sync is owned
                # by W2 transposes; a bank-slot wait here is harmless)
                nc.scalar.dma_start(
                    bank_t[:].rearrange("p (j f) -> p j f", f=128),
                    xq[:].rearrange("p (j f) -> p j f", f=128),
                    transpose=True,
                )
                banks[tt] = bank_t
                if with_sy:
                    emit_sy(tt)

            # ---- W1: dual-queue loads, |w| row sums on DVE ----
            w1_engs = [nc.gpsimd, nc.sync, nc.scalar]
            for ot in range(OT):
                wt = wstage.tile([128, K], F32, tag="wst", name=f"wt_{ot}")
                w1_engs[ot % 3].dma_start(wt[:], w[ot * 128 : (ot + 1) * 128, :])
                nc.vector.tensor_reduce(
                    wsum[:, ot : ot + 1],
                    wt[:],
                    axis=mybir.AxisListType.X,
                    op=ALU.add,
                    apply_absolute_value=True,
                )

            # ---- alpha reduce + AllReduce (sync queue) ----
            wred = spool.tile([128, 1], F32, tag="wred")
            nc.vector.reduce_sum(wred[:], wsum[:], axis=mybir.AxisListType.X)
            pss = ps.tile([1, 1], F32, tag="ps", name="pss")
            nc.tensor.matmul(pss[:], wred[:], ones_col[:], start=True, stop=True)
            total_sb = spool.tile([1, 8], F32, tag="total_sb")
            nc.vector.memset(total_sb[:], 0.0)
            nc.vector.tensor_copy(total_sb[:, 0:1], pss[:])

            cc_in = dram.tile([1, 8], F32, tag="cc_in")
            cc_out = dram.tile([1, 8], F32, tag="cc_out")
            nc.gpsimd.dma_start(cc_in[:], total_sb[:])
            nc.gpsimd.collective_compute(
                "AllReduce",
                ALU.add,
                replica_groups=[list(range(n_cores))],
                ins=[cc_in.opt()],
                outs=[cc_out.opt()],
            )

            # W2 prefetch: first ring-depth reloads transfer during the
            # CC wait (gpsimd is blocked by the CC; scalar+sync are free)
            w2_order = list(range(OT))
            w2_tiles = {}
            for ot in w2_order[: min(ring, OT)]:
                wt2 = wstage.tile([128, K], F32, tag="wst", name=f"wt2_{ot}")
                nc.scalar.dma_start(wt2[:], w[ot * 128 : (ot + 1) * 128, :])
                w2_tiles[ot] = wt2

            # chains 0..pre-1 run during the collective window
            for t in range(min(pre, TT)):
                chain(t, with_sy=False)

            gtot = spool.tile([1, 1], F32, tag="gtot")
            nc.gpsimd.dma_start(gtot[:], cc_out[:, 0:1])
            alpha_s = spool.tile([1, 1], F32, tag="alpha_s")
            nc.vector.tensor_scalar(
                out=alpha_s[:],
                in0=gtot[:],
                scalar1=inv_count,
                scalar2=1e-10,
                op0=ALU.mult,
                op1=ALU.max,
            )
            # broadcast alpha to all 128 partitions with one matmul
            psa = ps.tile([128, 1], F32, tag="ps", name="psa")
            nc.tensor.matmul(psa[:], ones_row[:], alpha_s[:], start=True, stop=True)
            nc.vector.tensor_copy(alpha_bc[:], psa[:])
            nc.vector.reciprocal(inv_alpha_bc[:], alpha_bc[:])
            for t in range(min(pre, TT)):
                emit_sy(t)

            # ---- W2: quantize + transpose into kt-major fp8 ----
            wqs_r = wqs[:].rearrange("p (kt ot f) -> p kt ot f", kt=KT, f=128)

            def w2_tile(ot):
                if ot in w2_tiles:
                    wt2 = w2_tiles.pop(ot)
                else:
                    wt2 = wstage.tile([128, K], F32, tag="wst", name=f"wt2_{ot}")
                    eng = nc.gpsimd if ot % 2 == 0 else nc.sync
                    r0 = ot * 128
                    eng.dma_start(
                        wt2[:, : K // 2], w[r0 : r0 + 128, : K // 2]
                    )
                    eng.dma_start(
                        wt2[:, K // 2 :], w[r0 : r0 + 128, K // 2 :]
                    )
                for h in range(2):
                    sl = slice(h * (K // 2), (h + 1) * (K // 2))
                    # (w * 1/alpha) + MAGIC rounds to nearest int, in place
                    nc.vector.tensor_scalar(
                        out=wt2[:, sl],
                        in0=wt2[:, sl],
                        scalar1=inv_alpha_bc[:],
                        scalar2=MAGIC,
                        op0=ALU.mult,
                        op1=ALU.add,
                    )
                    wqb = w16p.tile(
                        [128, K // 2], BF16, tag="w16", name=f"wqb_{ot}_{h}"
                    )
                    nc.vector.tensor_scalar_add(wqb[:], wt2[:, sl], -MAGIC)
                    wqT = w16p.tile(
                        [128, K // 2], BF16, tag="w16T", name=f"wqT_{ot}_{h}"
                    )
                    nc.sync.dma_start(
                        wqT[:].rearrange("p (j f) -> p j f", f=128),
                        wqb[:].rearrange("p (j f) -> p j f", f=128),
                        transpose=True,
                    )
                    # both clips fused into the fp8 convert (kt-major write)
                    nc.vector.tensor_scalar(
                        out=wqs_r[:, h * (KT // 2) : (h + 1) * (KT // 2), ot, :],
                        in0=wqT[:].rearrange("p (kt f) -> p kt f", f=128),
                        scalar1=-1.0,
                        scalar2=1.0,
                        op0=ALU.max,
                        op1=ALU.min,
                    )

            # ---- matmuls ----
            def burst(tt, ob):
                bank_t = banks[tt]
                psum = ps.tile([128, OBW], F32, tag="ps", name=f"psum_{tt}_{ob}")
                for kt in range(KT):
                    nc.tensor.matmul(
                        psum[:],
                        bank_t[:, kt * 128 : (kt + 1) * 128],
                        wqs[:, kt * O + ob * OBW : kt * O + (ob + 1) * OBW],
                        start=(kt == 0),
                        stop=(kt == KT - 1),
                    )
                osb = osbp.tile([128, OBW], BF16, tag="osb", name=f"osb_{tt}_{ob}")
                nc.scalar.mul(osb[:], psum[:], sy_tiles[tt][:])
                nc.scalar.dma_start(
                    y[tt * 128 : (tt + 1) * 128, ob * OBW : (ob + 1) * OBW],
                    osb[:],
                )

            for ot in w2_order:
                w2_tile(ot)

            # phase A: first npA row-tiles ob-major (tracks W2 arrival)
            for ob in range(OBN):
                for tt in range(min(npA, TT)):
                    burst(tt, ob)
            for tt in range(min(npA, TT)):
                banks.pop(tt)
                sy_tiles.pop(tt)
            for t in range(npA, min(npA + nbank, TT)):
                chain(t, with_sy=True)

            # phase B: remaining row-tiles, tt-major
            for tt in range(npA, TT):
                for ob in range(OBN):
                    burst(tt, ob)
                banks.pop(tt)
                sy_tiles.pop(tt)
                nxt = tt + nbank
                if npA + nbank <= nxt < TT:
                    chain(nxt, with_sy=True)

    return nc


_nc_cache = {}


def _get_nc(T, K, O, n_cores, use_nw):
    key = (T, K, O, n_cores, use_nw)
    if key not in _nc_cache:
        nc = build(T, K, O, n_cores, use_nw)
        _split_sync_waits(nc)
        _nc_cache[key] = nc
    return _nc_cache[key]


def kernel(x: np.ndarray, weight: np.ndarray, norm_weight: np.ndarray) -> np.ndarray:
    B, S, K = x.shape
    T = B * S
    Ofull, _ = weight.shape
    O = Ofull // N_CORES

    use_nw = not bool(np.all(norm_weight == 1.0))
    nc = _get_nc(T, K, O, N_CORES, use_nw)

    xf = np.ascontiguousarray(x.reshape(T, K).astype(np.float32, copy=False))
    nwf = np.ascontiguousarray(
        norm_weight.reshape(1, K).astype(np.float32, copy=False)
    )
    in_maps = [
        {
            "x": xf,
            "w": np.ascontiguousarray(weight[i * O : (i + 1) * O]),
            "nw": nwf,
        }
        for i in range(N_CORES)
    ]
    res = run_bass_kernel_spmd(nc, in_maps, list(range(N_CORES))).results
    y = np.concatenate(
        [res[i]["y"].astype(np.float32) for i in range(N_CORES)], axis=1
    )
    return y.reshape(B, S, Ofull)


# revision 4
# speedup vs baseline: 1.0764x; 1.0764x over previous
"""BitLinear forward, tensor-parallel over 8 NeuronCores.

Sharding as v1 (weight rows split 8 ways, x replicated, host concat).

Schedule:
- W1: pure dual-queue (gpsimd+sync) streaming of the 32MB w shard through
  a 3-tile ring, |w| row sums on DVE. No other DMA traffic in this window.
- alpha AllReduce issued on the sync queue (idle during the collective);
  gpsimd/vector/scalar keep flowing: x-quant chains 0-2 (x loads on the
  scalar queue) and the first W2 reloads prefetch during the CC wait.
- W2: quantize on DVE (magic-round mul, clip, fused min+fp8 convert),
  transposes on sync, loads alternating gpsimd/vector, into a kt-major
  fp8 layout (every matmul moving operand is a contiguous [128,512]).
- Matmuls start when o-tiles 0-3 are ready; the first 3 row-tiles run
  ob-major (sweep o-blocks across tts) so consumption tracks W2's
  o-tile arrival order with no stalls.
- Output stored as bf16 (halves store traffic; host upcasts — the
  rounding is ~2^-9 rel, far under the 2e-2 gate).
"""

import numpy as np

import concourse.bass as bass
import concourse.mybir as mybir
import concourse.tile as tile
from concourse.bass_utils import run_bass_kernel_spmd


MAX_ATTACHED_WAITS = 1


def _split_sync_waits(nc, max_waits=MAX_ATTACHED_WAITS):
    nhoisted = 0
    for f in nc.m.functions:
        for blk in f.blocks:
            out = []
            changed = False
            for inst in blk.instructions:
                si = inst.sync_info
                if si is not None and len(si.on_wait) > max_waits:
                    waits = list(si.on_wait)
                    for wt in waits[max_waits:]:
                        out.append(
                            mybir.InstNoOp(
                                name=f"syncsplit-{nc.next_id()}",
                                ins=[],
                                outs=[],
                                engine=inst.engine,
                                sync_info=mybir.SyncInfo(
                                    on_wait=[wt], on_update=[]
                                ),
                                bass_nofuse=True,
                            )
                        )
                        nhoisted += 1
                    inst.sync_info = mybir.SyncInfo(
                        on_wait=waits[:max_waits], on_update=list(si.on_update)
                    )
                    changed = True
                out.append(inst)
            if changed:
                blk.instructions = out
    return nhoisted


F32 = mybir.dt.float32
BF16 = mybir.dt.bfloat16
FP8 = mybir.dt.float8e4

MAGIC = 1.5 * 2.0**23  # add/sub rounds f32 to nearest int (ties to even)
EPS = 1e-6

N_CORES = 8
AFT = mybir.ActivationFunctionType
ALU = mybir.AluOpType


def build(T, K, O, n_cores, use_nw, ring=3, nbank=3, pre=3, npA=3):
    """One-core SPMD program: x[T,K] f32, w[O,K] f32 shard, nw[1,K] -> y[T,O]."""
    TT, KT, OT = T // 128, K // 128, O // 128
    OBN = max(1, O // 512)
    OBW = O // OBN
    assert OBW <= 512
    assert pre == nbank

    nc = bass.Bass(
        "TRN2", target_bir_lowering=False, debug=False, num_devices=n_cores
    )
    x = nc.dram_tensor("x", [T, K], F32, kind="ExternalInput")
    w = nc.dram_tensor("w", [O, K], F32, kind="ExternalInput")
    nw = nc.dram_tensor("nw", [1, K], F32, kind="ExternalInput")
    y = nc.dram_tensor("y", [T, O], BF16, kind="ExternalOutput")

    inv_count = 1.0 / (O * n_cores * K)

    with tile.TileContext(nc) as tc:
        with (
            tc.tile_pool(name="const", bufs=1) as cpool,
            tc.tile_pool(name="wres", bufs=1) as wres,
            tc.tile_pool(name="wstage", bufs=ring) as wstage,
            tc.tile_pool(name="xstage", bufs=2) as xstage,
            tc.tile_pool(name="xq", bufs=2) as xqp,
            tc.tile_pool(name="bank", bufs=nbank) as bankp,
            tc.tile_pool(name="w16", bufs=1) as w16p,
            tc.tile_pool(name="osb", bufs=2) as osbp,
            tc.tile_pool(name="stat", bufs=4) as spool,
            tc.tile_pool(name="psum", bufs=8, space="PSUM") as ps,
            tc.tile_pool(name="dram", bufs=1, space="DRAM") as dram,
        ):
            # ---- constants ----
            posmagic = cpool.tile([128, 1], F32, tag="posmagic")
            nc.vector.memset(posmagic[:], MAGIC)
            epsb = cpool.tile([128, 1], F32, tag="epsb")
            nc.vector.memset(epsb[:], EPS)
            ones_col = cpool.tile([128, 1], F32, tag="ones_col")
            nc.vector.memset(ones_col[:], 1.0)
            ones_row = cpool.tile([1, 128], F32, tag="ones_row")
            nc.vector.memset(ones_row[:], 1.0)
            alpha_bc = cpool.tile([128, 1], F32, tag="alpha_bc")
            inv_alpha_bc = cpool.tile([128, 1], F32, tag="inv_alpha_bc")
            wsum = cpool.tile([128, OT], F32, tag="wsum")

            if use_nw:
                nw_rep = cpool.tile([128, K], F32, tag="nw_rep")
                nc.gpsimd.dma_start(nw_rep[0:1, :], nw.ap())
                p = 1
                while p < 128:
                    nc.gpsimd.dma_start(nw_rep[p : 2 * p, :], nw_rep[0:p, :])
                    p *= 2

            # resident fp8 weights, kt-major: column kt*O + o holds
            # w_q[o, kt*128 + p]
            wqs = wres.tile([128, KT * O], FP8, tag="wqs")

            sys_ = {}
            banks = {}
            sy_tiles = {}

            def emit_sy(tt):
                gor = sys_.pop(tt)
                sy = spool.tile([128, 1], F32, tag="sy", name=f"sy_{tt}")
                nc.vector.tensor_scalar(
                    out=sy[:],
                    in0=gor[:],
                    scalar1=alpha_bc[:],
                    scalar2=1.0 / 127.0,
                    op0=ALU.mult,
                    op1=ALU.mult,
                )
                sy_tiles[tt] = sy

            def chain(tt, with_sy):
                xin = xstage.tile([128, K], F32, tag="xin", name=f"xin_{tt}")
                nc.scalar.dma_start(xin[:], x[tt * 128 : (tt + 1) * 128, :])

                bank_t = bankp.tile([128, K], BF16, tag="bank", name=f"bank_{tt}")
                ss = spool.tile([128, 1], F32, tag="ss", name=f"ss_{tt}")
                # elementwise square output is scratch; dump into the bank
                # tile (overwritten below by the transpose)
                nc.scalar.activation(
                    bank_t[:], xin[:], AFT.Square, accum_out=ss[:]
                )

                if use_nw:
                    u = xstage.tile([128, K], F32, tag="xin", name=f"u_{tt}")
                    nc.vector.tensor_mul(u[:], xin[:], nw_rep[:])
                    src = u
                else:
                    src = xin

                graw = spool.tile([128, 1], F32, tag="graw", name=f"graw_{tt}")
                nc.vector.tensor_reduce(
                    graw[:],
                    src[:],
                    axis=mybir.AxisListType.X,
                    op=ALU.max,
                    apply_absolute_value=True,
                )
                g = spool.tile([128, 1], F32, tag="g", name=f"g_{tt}")
                nc.vector.tensor_scalar_max(g[:], graw[:], 1e-10)
                invg = spool.tile([128, 1], F32, tag="invg", name=f"invg_{tt}")
                nc.vector.reciprocal(invg[:], g[:])
                s127 = spool.tile([128, 1], F32, tag="s127", name=f"s127_{tt}")
                nc.vector.tensor_scalar_mul(s127[:], invg[:], 127.0)
                rms = spool.tile([128, 1], F32, tag="rms", name=f"rms_{tt}")
                nc.scalar.activation(
                    rms[:], ss[:], AFT.Sqrt, bias=epsb[:], scale=1.0 / K
                )
                invrms = spool.tile([128, 1], F32, tag="invrms", name=f"invrms_{tt}")
                nc.vector.reciprocal(invrms[:], rms[:])
                gor = spool.tile([128, 1], F32, tag="gor", name=f"gor_{tt}")
                nc.vector.tensor_mul(gor[:], g[:], invrms[:])
                sys_[tt] = gor

                # q1 = src*127/g + MAGIC, in place over the staging tile
                nc.vector.tensor_scalar(
                    out=src[:],
                    in0=src[:],
                    scalar1=s127[:],
                    scalar2=MAGIC,
                    op0=ALU.mult,
                    op1=ALU.add,
                )
                xq = xqp.tile([128, K], BF16, tag="xq", name=f"xq_{tt}")
                nc.vector.tensor_scalar_add(xq[:], src[:], -MAGIC)

                # transpose into the bank tile; scalar queue (sync is owned
                # by W2 transposes; a bank-slot wait here is harmless)
                nc.scalar.dma_start(
                    bank_t[:].rearrange("p (j f) -> p j f", f=128),
                    xq[:].rearrange("p (j f) -> p j f", f=128),
                    transpose=True,
                )
                banks[tt] = bank_t
                if with_sy:
                    emit_sy(tt)

            # ---- W1: dual-queue loads, |w| row sums on DVE ----
            w1_engs = [nc.gpsimd, nc.sync, nc.scalar]
            for ot in range(OT):
                wt = wstage.tile([128, K], F32, tag="wst", name=f"wt_{ot}")
                w1_engs[ot % 3].dma_start(wt[:], w[ot * 128 : (ot + 1) * 128, :])
                nc.vector.tensor_reduce(
                    wsum[:, ot : ot + 1],
                    wt[:],
                    axis=mybir.AxisListType.X,
                    op=ALU.add,
                    apply_absolute_value=True,
                )

            # ---- alpha reduce + AllReduce (sync queue) ----
            wred = spool.tile([128, 1], F32, tag="wred")
            nc.vector.reduce_sum(wred[:], wsum[:], axis=mybir.AxisListType.X)
            pss = ps.tile([1, 1], F32, tag="ps", name="pss")
            nc.tensor.matmul(pss[:], wred[:], ones_col[:], start=True, stop=True)
            total_sb = spool.tile([1, 8], F32, tag="total_sb")
            nc.vector.memset(total_sb[:], 0.0)
            nc.vector.tensor_copy(total_sb[:, 0:1], pss[:])

            cc_in = dram.tile([1, 8], F32, tag="cc_in")
            cc_out = dram.tile([1, 8], F32, tag="cc_out")
            nc.gpsimd.dma_start(cc_in[:], total_sb[:])
            nc.gpsimd.collective_compute(
                "AllReduce",
                ALU.add,
                replica_groups=[list(range(n_cores))],
                ins=[cc_in.opt()],
                outs=[cc_out.opt()],
            )

            # W2 prefetch: first ring-depth reloads transfer during the
            # CC wait (gpsimd is blocked by the CC; scalar+sync are free)
            w2_order = list(range(OT))
            w2_tiles = {}
            for ot in w2_order[: min(ring, OT)]:
                wt2 = wstage.tile([128, K], F32, tag="wst", name=f"wt2_{ot}")
                nc.scalar.dma_start(wt2[:], w[ot * 128 : (ot + 1) * 128, :])
                w2_tiles[ot] = wt2

            # chains 0..pre-1 run during the collective window
            for t in range(min(pre, TT)):
                chain(t, with_sy=False)

            gtot = spool.tile([1, 1], F32, tag="gtot")
            nc.gpsimd.dma_start(gtot[:], cc_out[:, 0:1])
            alpha_s = spool.tile([1, 1], F32, tag="alpha_s")
            nc.vector.tensor_scalar(
                out=alpha_s[:],
                in0=gtot[:],
                scalar1=inv_count,
                scalar2=1e-10,
                op0=ALU.mult,
                op1=ALU.max,
            )
            # broadcast alpha to all 128 partitions with one matmul
            psa = ps.tile([128, 1], F32, tag="ps", name="psa")
            nc.tensor.matmul(psa[:], ones_row[:], alpha_s[:], start=True, stop=True)
            nc.vector.tensor_copy(alpha_bc[:], psa[:])
            nc.vector.reciprocal(inv_alpha_bc[:], alpha_bc[:])
            for t in range(min(pre, TT)):
                emit_sy(t)

            # ---- W2: quantize + transpose into kt-major fp8 ----
            wqs_r = wqs[:].rearrange("p (kt ot f) -> p kt ot f", kt=KT, f=128)

            def w2_tile(ot):
                if ot in w2_tiles:
                    wt2 = w2_tiles.pop(ot)
                else:
                    wt2 = wstage.tile([128, K], F32, tag="wst", name=f"wt2_{ot}")
                    eng = nc.gpsimd if ot % 2 == 0 else nc.sync
                    eng.dma_start(wt2[:], w[ot * 128 : (ot + 1) * 128, :])
                # (w * 1/alpha) + MAGIC rounds to nearest int, in place
                nc.vector.tensor_scalar(
                    out=wt2[:],
                    in0=wt2[:],
                    scalar1=inv_alpha_bc[:],
                    scalar2=MAGIC,
                    op0=ALU.mult,
                    op1=ALU.add,
                )
                wqb = w16p.tile([128, K], BF16, tag="w16", name=f"wqb_{ot}")
                nc.vector.tensor_scalar(
                    out=wqb[:],
                    in0=wt2[:],
                    scalar1=MAGIC,
                    scalar2=-1.0,
                    op0=ALU.subtract,
                    op1=ALU.max,
                )
                for h in range(2):
                    wqT = w16p.tile(
                        [128, K // 2], BF16, tag="w16T", name=f"wqT_{ot}_{h}"
                    )
                    nc.sync.dma_start(
                        wqT[:].rearrange("p (j f) -> p j f", f=128),
                        wqb[:, h * (K // 2) : (h + 1) * (K // 2)].rearrange(
                            "p (j f) -> p j f", f=128
                        ),
                        transpose=True,
                    )
                    # fused upper clip + fp8 convert into kt-major layout
                    nc.vector.tensor_scalar_min(
                        wqs_r[:, h * (KT // 2) : (h + 1) * (KT // 2), ot, :],
                        wqT[:].rearrange("p (kt f) -> p kt f", f=128),
                        1.0,
                    )

            # ---- matmuls ----
            def burst(tt, ob):
                bank_t = banks[tt]
                psum = ps.tile([128, OBW], F32, tag="ps", name=f"psum_{tt}_{ob}")
                for kt in range(KT):
                    nc.tensor.matmul(
                        psum[:],
                        bank_t[:, kt * 128 : (kt + 1) * 128],
                        wqs[:, kt * O + ob * OBW : kt * O + (ob + 1) * OBW],
                        start=(kt == 0),
                        stop=(kt == KT - 1),
                    )
                osb = osbp.tile([128, OBW], BF16, tag="osb", name=f"osb_{tt}_{ob}")
                nc.scalar.mul(osb[:], psum[:], sy_tiles[tt][:])
                nc.scalar.dma_start(
                    y[tt * 128 : (tt + 1) * 128, ob * OBW : (ob + 1) * OBW],
                    osb[:],
                )

            for ot in w2_order:
                w2_tile(ot)

            # phase A: first npA row-tiles ob-major (tracks W2 arrival)
            for ob in range(OBN):
                for tt in range(min(npA, TT)):
                    burst(tt, ob)
            for tt in range(min(npA, TT)):
                banks.pop(tt)
                sy_tiles.pop(tt)
            for t in range(npA, min(npA + nbank, TT)):
                chain(t, with_sy=True)

            # phase B: remaining row-tiles, tt-major
            for tt in range(npA, TT):
                for ob in range(OBN):
                    burst(tt, ob)
                banks.pop(tt)
                sy_tiles.pop(tt)
                nxt = tt + nbank
                if npA + nbank <= nxt < TT:
                    chain(nxt, with_sy=True)

    return nc


_nc_cache = {}


def _get_nc(T, K, O, n_cores, use_nw):
    key = (T, K, O, n_cores, use_nw)
    if key not in _nc_cache:
        nc = build(T, K, O, n_cores, use_nw)
        _split_sync_waits(nc)
        _nc_cache[key] = nc
    return _nc_cache[key]


def kernel(x: np.ndarray, weight: np.ndarray, norm_weight: np.ndarray) -> np.ndarray:
    B, S, K = x.shape
    T = B * S
    Ofull, _ = weight.shape
    O = Ofull // N_CORES

    use_nw = not bool(np.all(norm_weight == 1.0))
    nc = _get_nc(T, K, O, N_CORES, use_nw)

    xf = np.ascontiguousarray(x.reshape(T, K).astype(np.float32, copy=False))
    nwf = np.ascontiguousarray(
        norm_weight.reshape(1, K).astype(np.float32, copy=False)
    )
    in_maps = [
        {
            "x": xf,
            "w": np.ascontiguousarray(weight[i * O : (i + 1) * O]),
            "nw": nwf,
        }
        for i in range(N_CORES)
    ]
    res = run_bass_kernel_spmd(nc, in_maps, list(range(N_CORES))).results
    y = np.concatenate(
        [res[i]["y"].astype(np.float32) for i in range(N_CORES)], axis=1
    )
    return y.reshape(B, S, Ofull)
